# revision 26
# baseline (speedup 1.0000x reference)
"""Trainium2 Bass kernel for nn_BaseLinearSSM.

y[b,t] = Re(C @ x_{t+1}) + D @ u[b,t] + bias,  x_{t+1} = A x_t + B u_t  (complex A,B,C)

Strategy (v4, hybrid fp16, even/odd decimated scan):
  Host (fp64): eigendecompose A = V diag(w) V^-1, fold V into B/C:
  Bt = V^-1 B, Ct = C V.  Sort modes by |w| descending and split:

  * LONG modes (top NL=256, two 128-partition tiles, balanced ||bt_n||=1):
    decimate time by 2.  With X_k = state after consuming u_{2k+1}:
        X_k = w^2 X_{k-1} + h_k,   h_k = w f_{2k} + f_{2k+1}
    The X recurrence runs through the standard modulated-scan machinery
    (tables at 2*theta, decay rho^2) at HALF the time resolution -- this
    halves all DVE elementwise work, the only non-matmul cost.  Outputs:
        y_{2k+1} = Re(Ct X_k)          (products of X_k, lhsT Ct)
        y_{2k}   = Re(Ct w X_{k-1}) + Re(Ct Bt) u_{2k}
    i.e. odd/even y columns are STATIONARY matmuls on the demodulated
    products (lhsT Ct resp. Ct*w), written with stride-2 column APs into
    the y PSUM banks.  h is built during the PSUM->SBUF eviction: the
    scalar engine applies the per-partition scale w_re/w_im to the even
    f columns while copying (out = in*scale), so the DVE only adds.

  * SHORT modes (|w| < ~0.64): 14-lag convolution y += sum_k Re(Ct w^k
    Bt) u_{t-k} on the tensor engine (fp16 [128,128] kernels, D at lag 0).

  Numerics validated host-side: fp16 hybrid rel err ~4e-3 (gate 2e-2).
  Cores independent (weights replicated); host shards u, gathers y.
"""

import sys

import numpy as np

if "/opt/trn_rl_repo" not in sys.path:
    sys.path.insert(0, "/opt/trn_rl_repo")

BATCH, T, IN, OUT, N = 16, 2048, 128, 128, 512
NCORES = 8
BLOCAL = BATCH // NCORES  # 2
COLS = BLOCAL * T         # 4096 columns per core
BLK = 512                 # f-matmul / conv sub-block columns
DBLK = 1024               # double-block: columns per pipeline stage
NDB = COLS // DBLK        # 4 double-blocks (2 per batch element)
DPB = NDB // BLOCAL       # 2 double-blocks per batch element
HBLK = DBLK // 2          # 512 X-steps per double-block
NL = 256                  # long modes (scan path)
LT = NL // 128            # 2 long-mode partition tiles
KCONV = 12                # conv lags for short modes (incl. D at lag 0)

W_UT = COLS
W_CONV = KCONV * OUT
W_B = LT * 128
SETW = 2 * LT * HBLK      # one set's cos+sin tables at 2*theta ([128, 2048])
W_CT = LT * OUT
RHOW = LT * HBLK          # rho^2 table
BLOBW = (W_UT + W_CONV + 4 * W_B + 4 * W_CT + OUT + RHOW + DPB * SETW)

LAST_RESULT = None
_NC_CACHE = None


def _build_nc():
    from concourse import bass, mybir
    from concourse import tile

    f16 = mybir.dt.float16
    f32 = mybir.dt.float32
    op = mybir.AluOpType
    ACT = mybir.ActivationFunctionType

    nc = bass.Bass("TRN2", target_bir_lowering=False, debug=False)

    blob = nc.dram_tensor("blob", [128, BLOBW], f16, kind="ExternalInput")
    yout = nc.dram_tensor("y", [OUT, COLS], f16, kind="ExternalOutput")

    with tile.TileContext(nc) as tc:
        with (
            tc.tile_pool(name="const", bufs=1) as cpool,
            tc.tile_pool(name="f16", bufs=2) as fspool,
            tc.tile_pool(name="tmp", bufs=1) as tpool,
            tc.tile_pool(name="gp", bufs=1) as gpool,
            tc.tile_pool(name="zp", bufs=2) as zpool,
            tc.tile_pool(name="dm", bufs=2) as dpool,
            tc.tile_pool(name="ysb", bufs=2) as spool,
            tc.tile_pool(name="fps", bufs=1, space="PSUM") as fpool,
            tc.tile_pool(name="yps", bufs=2, space="PSUM") as ypool,
        ):
            blob_sb = cpool.tile([128, BLOBW], f16)
            for a, bnd in [(0, 1024), (1024, 2048), (2048, W_UT)]:
                nc.gpsimd.dma_start(blob_sb[:, a:bnd], blob[:, a:bnd])
            W0 = W_UT
            W1 = W0 + 4 * W_B                         # btr+bti+wbr+wbi
            W2 = W1 + SETW + RHOW                     # set0 tables + rho2
            W25 = W2 + W_CONV                         # convw
            W3 = W25 + 4 * W_CT + OUT                 # ctr,cti,ctwr,ctwin,k0l
            for a, bnd in [(W0, W1), (W1, W2), (W2, W25), (W25, W3),
                           (W3, W3 + SETW)]:
                nc.sync.dma_start(blob_sb[:, a:bnd], blob[:, a:bnd])

            o = [0]
            def take(w):
                s = blob_sb[:, o[0]:o[0] + w]
                o[0] += w
                return s
            ut_sb = take(W_UT)
            btr_sb = take(W_B)
            bti_sb = take(W_B)
            wbr_sb = take(W_B)
            wbi_sb = take(W_B)
            ct_set = [None] * DPB
            st_set = [None] * DPB
            ct_set[0] = take(LT * HBLK)
            st_set[0] = take(LT * HBLK)
            rho_sb = take(RHOW)
            convw = take(W_CONV)
            ctr_sb = take(W_CT)
            cti_sb = take(W_CT)       # holds -Ct.imag
            ctwr_sb = take(W_CT)
            ctwin_sb = take(W_CT)     # holds -(Ct w).imag
            k0l_sb = take(OUT)
            for k in range(1, DPB):
                ct_set[k] = take(LT * HBLK)
                st_set[k] = take(LT * HBLK)
            assert o[0] == BLOBW

            WH = LT * HBLK  # 1024: wide elementwise op width

            def emit_h(db):
                """PE: h = (w Bt) u_even + Bt u_odd directly in PSUM
                (pair-fold folded into the input matrices host-side);
                ACT: evict h to fp16 SBUF."""
                b, dbt = divmod(db, DPB)
                col0 = b * T + 2 * dbt * BLK
                ue = ut_sb[:, col0:col0 + DBLK:2]
                uo = ut_sb[:, col0 + 1:col0 + DBLK:2]
                hr16 = fspool.tile([128, WH], f16, tag="hr16")
                hi16 = fspool.tile([128, WH], f16, tag="hi16")
                for m in range(LT):
                    msl = slice(m * 128, (m + 1) * 128)
                    hpr = fpool.tile([128, HBLK], f32, tag=f"hr{m}")
                    hpi = fpool.tile([128, HBLK], f32, tag=f"hi{m}")
                    nc.tensor.matmul(hpr[:], wbr_sb[:, msl], ue, start=True, stop=False)
                    nc.tensor.matmul(hpr[:], btr_sb[:, msl], uo, start=False, stop=True)
                    nc.tensor.matmul(hpi[:], wbi_sb[:, msl], ue, start=True, stop=False)
                    nc.tensor.matmul(hpi[:], bti_sb[:, msl], uo, start=False, stop=True)
                    dst = slice(m * HBLK, (m + 1) * HBLK)
                    nc.scalar.copy(hr16[:, dst], hpr[:])
                    nc.scalar.copy(hi16[:, dst], hpi[:])
                yps = ypool.tile([128, DBLK], f32, tag="y")
                return yps, (hr16, hi16)

            def emit_conv(db, yps, sub):
                """PE: conv for one 512-col sub-block of double-block db,
                plus (on sub 1) the K0L term on even columns."""
                b, dbt = divmod(db, DPB)
                col0 = b * T + 2 * dbt * BLK
                c0 = col0 + sub * BLK
                ysl = slice(sub * BLK, (sub + 1) * BLK)
                for k in range(KCONV):
                    wk = convw[:, k * OUT:(k + 1) * OUT]
                    if dbt == 0 and sub == 0 and k > 0:
                        nc.tensor.matmul(
                            yps[:, sub * BLK + k:(sub + 1) * BLK], wk,
                            ut_sb[:, c0:c0 + BLK - k],
                            start=False, stop=False,
                        )
                    else:
                        nc.tensor.matmul(
                            yps[:, ysl], wk, ut_sb[:, c0 - k:c0 + BLK - k],
                            start=(k == 0), stop=False,
                        )
                if sub == 1:
                    # K0L on even columns -- must come after both start=True
                    # conv lag-0 matmuls have initialised the banks
                    nc.tensor.matmul(
                        yps[:, 0:DBLK:2], k0l_sb[:],
                        ut_sb[:, col0:col0 + DBLK:2],
                        start=False, stop=False,
                    )

            zprev = [None, None]

            def emit_mod(db, ev):
                """DVE: modulate h by e^{-i 2th k}."""
                hr16, hi16 = ev
                dbt = db % DPB
                ctt, stt = ct_set[dbt][:], st_set[dbt][:]
                t1 = tpool.tile([128, WH], f16, tag="t1")
                t2 = tpool.tile([128, WH], f16, tag="t2")
                t3 = tpool.tile([128, WH], f16, tag="t3")
                t4 = tpool.tile([128, WH], f16, tag="t4")
                nc.vector.tensor_tensor(t1[:], ctt, hr16[:], op=op.mult)
                nc.vector.tensor_tensor(t2[:], stt, hi16[:], op=op.mult)
                nc.vector.tensor_tensor(t3[:], ctt, hi16[:], op=op.mult)
                nc.vector.tensor_tensor(t4[:], stt, hr16[:], op=op.mult)
                return t1, t2, t3, t4

            def emit_adds_scans(db, t1, t2, t3, t4):
                dbt = db % DPB
                gr = gpool.tile([128, WH], f16, tag="gr")
                gi = gpool.tile([128, WH], f16, tag="gi")
                nc.vector.tensor_tensor(gr[:], t1[:], t2[:], op=op.add)
                nc.vector.tensor_tensor(gi[:], t3[:], t4[:], op=op.subtract)
                zr = zpool.tile([128, WH], f16, tag="zr")
                zi = zpool.tile([128, WH], f16, tag="zi")
                for m in range(LT):
                    sl = slice(m * HBLK, (m + 1) * HBLK)
                    last = m * HBLK + HBLK - 1
                    init_r = 0.0 if dbt == 0 else zprev[0][:, last:last + 1]
                    init_i = 0.0 if dbt == 0 else zprev[1][:, last:last + 1]
                    nc.vector.tensor_tensor_scan(
                        zr[:, sl], rho_sb[:, sl], gr[:, sl], init_r,
                        op0=op.mult, op1=op.add,
                    )
                    nc.vector.tensor_tensor_scan(
                        zi[:, sl], rho_sb[:, sl], gi[:, sl], init_i,
                        op0=op.mult, op1=op.add,
                    )
                zprev[0], zprev[1] = zr, zi
                return zr, zi

            def emit_demod_tile(db, m, zr, zi):
                dbt = db % DPB
                sl = slice(m * HBLK, (m + 1) * HBLK)
                ctt, stt = ct_set[dbt][:, sl], st_set[dbt][:, sl]
                p = dpool.tile([128, HBLK], f16, tag=f"p{m}")
                q = dpool.tile([128, HBLK], f16, tag=f"q{m}")
                r = dpool.tile([128, HBLK], f16, tag=f"r{m}")
                w2 = dpool.tile([128, HBLK], f16, tag=f"w2{m}")
                nc.vector.tensor_tensor(p[:], ctt, zr[:, sl], op=op.mult)
                nc.vector.tensor_tensor(q[:], stt, zr[:, sl], op=op.mult)
                nc.vector.tensor_tensor(r[:], stt, zi[:, sl], op=op.mult)
                nc.vector.tensor_tensor(w2[:], ctt, zi[:, sl], op=op.mult)
                pr = dpool.tile([128, HBLK], f16, tag=f"pr{m}")
                qw = dpool.tile([128, HBLK], f16, tag=f"qw{m}")
                nc.vector.tensor_tensor(pr[:], p[:], r[:], op=op.subtract)
                nc.vector.tensor_tensor(qw[:], q[:], w2[:], op=op.add)
                return pr, qw

            prprev = [None, None, None, None]  # per-tile (pr, qw) of prev db

            def emit_y_tile(db, m, yps, pr, qw):
                b, dbt = divmod(db, DPB)
                od_out = yps[:, 1:DBLK:2]
                ctr = ctr_sb[:, m * OUT:(m + 1) * OUT]
                cti = cti_sb[:, m * OUT:(m + 1) * OUT]
                ctwr = ctwr_sb[:, m * OUT:(m + 1) * OUT]
                ctwin = ctwin_sb[:, m * OUT:(m + 1) * OUT]
                # odd cols: Re(Ct X_k)
                nc.tensor.matmul(od_out, ctr, pr[:], start=False, stop=False)
                nc.tensor.matmul(od_out, cti, qw[:], start=False, stop=False)
                # even cols k>=1: Re(Ct w X_{k-1})
                sh = slice(0, HBLK - 1)
                nc.tensor.matmul(
                    yps[:, 2:DBLK:2], ctwr, pr[:, sh], start=False, stop=False)
                nc.tensor.matmul(
                    yps[:, 2:DBLK:2], ctwin, qw[:, sh], start=False,
                    stop=(dbt == 0 and m == LT - 1), skip_group_check=True)
                # even col 0 boundary: X from previous double-block
                if dbt > 0:
                    lastc = slice(HBLK - 1, HBLK)
                    nc.tensor.matmul(
                        yps[:, 0:1], ctwr, prprev[2 * m][:, lastc],
                        start=False, stop=False)
                    nc.tensor.matmul(
                        yps[:, 0:1], ctwin, prprev[2 * m + 1][:, lastc],
                        start=False, stop=(m == LT - 1),
                        skip_group_check=True)

            def emit_y_tail(db, yps):
                b, dbt = divmod(db, DPB)
                col0 = b * T + 2 * dbt * BLK
                ysb = spool.tile([128, DBLK], f16, tag="ysb")
                nc.scalar.copy(ysb[:], yps[:])
                nc.gpsimd.dma_start(yout[:, col0:col0 + DBLK], ysb[:])

            # software-pipelined emission over double-blocks; conv split so
            # half of it lands after y(db) on the PE queue (keeps PE warm
            # while demod products arrive).
            stage = [emit_h(0)]
            emit_conv(0, stage[0][0], 0)
            emit_conv(0, stage[0][0], 1)
            ts = emit_mod(0, stage[0][1])
            z = emit_adds_scans(0, *ts)
            for db in range(NDB):
                if db + 1 < NDB:
                    stage.append(emit_h(db + 1))
                    emit_conv(db + 1, stage[db + 1][0], 0)
                    ts_n = emit_mod(db + 1, stage[db + 1][1])
                prq = []
                for m in range(LT):
                    pr_m, qw_m = emit_demod_tile(db, m, *z)
                    emit_y_tile(db, m, stage[db][0], pr_m, qw_m)
                    prq += [pr_m, qw_m]
                prprev[:] = prq
                emit_y_tail(db, stage[db][0])
                if db + 1 < NDB:
                    emit_conv(db + 1, stage[db + 1][0], 1)
                    z = emit_adds_scans(db + 1, *ts_n)

    _legalize_multi_waits(nc)
    return nc


def _legalize_multi_waits(nc):
    """This walrus build accepts a single sync wait per instruction; split
    any multi-wait instruction into same-engine single-wait NoOps + the
    original carrying the last wait (program order chains them)."""
    import bass_rust
    from concourse import mybir

    uid = [0]
    for fn in nc.m.functions:
        for bb in fn.blocks:
            insts = bb.instructions
            new = []
            changed = False
            for inst in insts:
                si = inst.sync_info
                if si is not None and len(si.on_wait) > 1:
                    waits = list(si.on_wait)
                    for w in waits[:-1]:
                        uid[0] += 1
                        new.append(mybir.InstNoOp(
                            name=f"mwsplit-{uid[0]}",
                            engine=inst.engine,
                            ins=[], outs=[],
                            sync_info=bass_rust.SyncInfo(on_wait=[w], on_update=[]),
                        ))
                    inst.sync_info = bass_rust.SyncInfo(
                        on_wait=[waits[-1]], on_update=list(si.on_update)
                    )
                    changed = True
                new.append(inst)
            if changed:
                bb.instructions = new


def _host_prep(A_re, A_im, B_re, B_im, C_re, C_im, D_w):
    """fp64 eigendecomposition, mode sort/split, fp16 table/weight layouts."""
    A = A_re.astype(np.float64) + 1j * A_im.astype(np.float64)
    w, V = np.linalg.eig(A)
    Vinv = np.linalg.inv(V)
    Bt = Vinv @ (B_re.astype(np.float64) + 1j * B_im.astype(np.float64))
    Ct = (C_re.astype(np.float64) + 1j * C_im.astype(np.float64)) @ V
    rho_all = np.abs(w)
    order = np.argsort(-rho_all)
    li, si = order[:NL], order[NL:]

    bn = np.linalg.norm(Bt[li], axis=1)
    Btl = Bt[li] / bn[:, None]
    Ctl = Ct[:, li] * bn[None, :]
    wl = w[li]
    theta2 = 2.0 * np.angle(wl)
    rho2 = np.abs(wl) ** 2

    ws = w[si]
    convs = []
    for k in range(KCONV):
        M = ((Ct[:, si] * (ws ** k)) @ Bt[si]).real
        if k == 0:
            M = M + D_w.astype(np.float64)
        convs.append(np.ascontiguousarray(M.T, dtype=np.float16))

    # tables over X-steps k = 0..T/2-1 at angle 2*theta*(k+1)
    T2 = T // 2
    kg = np.arange(1, T2 + 1, dtype=np.float64)
    ang = np.outer(theta2, kg)  # [NL, T2]
    cost = np.cos(ang).astype(np.float16).reshape(LT, 128, T2)
    sint = np.sin(ang).astype(np.float16).reshape(LT, 128, T2)
    rho_b = np.broadcast_to(
        rho2.astype(np.float16).reshape(LT, 128, 1), (LT, 128, HBLK)
    ).transpose(1, 0, 2).reshape(128, LT * HBLK).copy()

    def setpiece(s):
        cs = cost[:, :, s * HBLK:(s + 1) * HBLK]
        ss = sint[:, :, s * HBLK:(s + 1) * HBLK]
        return [np.ascontiguousarray(cs.transpose(1, 0, 2).reshape(128, LT * HBLK)),
                np.ascontiguousarray(ss.transpose(1, 0, 2).reshape(128, LT * HBLK))]

    Ctw = Ctl * wl[None, :]
    def ct_part(M):  # [OUT, NL] -> [128, LT*OUT] lhsT layout
        MT = np.ascontiguousarray(M.T, dtype=np.float16)  # [NL, OUT]
        return np.ascontiguousarray(
            MT.reshape(LT, 128, OUT).transpose(1, 0, 2).reshape(128, LT * OUT))

    k0l = np.ascontiguousarray((Ctl @ Btl).real.T, dtype=np.float16)  # [IN, OUT]

    wBtl = wl[:, None] * Btl
    parts = [
        np.ascontiguousarray(Btl.real.T, dtype=np.float16),
        np.ascontiguousarray(Btl.imag.T, dtype=np.float16),
        np.ascontiguousarray(wBtl.real.T, dtype=np.float16),
        np.ascontiguousarray(wBtl.imag.T, dtype=np.float16),
    ]
    parts += setpiece(0)
    parts += [rho_b]
    parts += [np.concatenate(convs, axis=1)]
    parts += [ct_part(Ctl.real), ct_part(-Ctl.imag),
              ct_part(Ctw.real), ct_part(-Ctw.imag), k0l]
    for s in range(1, DPB):
        parts += setpiece(s)
    shared16 = np.concatenate(parts, axis=1)
    return shared16


def _ensure_axon_hooks():
    import types
    try:
        from antenv import axon_hooks  # noqa: F401
        return
    except ImportError:
        pass
    try:
        import antenv
        mod = types.ModuleType("antenv.axon_hooks")
        _hook = [None]
        mod.set_axon_ntff_profile_hook = lambda h: _hook.__setitem__(0, h)
        mod.get_axon_ntff_profile_hook = lambda: _hook[0]
        sys.modules["antenv.axon_hooks"] = mod
        antenv.axon_hooks = mod
        if "/root/.axon_site" not in sys.path:
            sys.path.insert(0, "/root/.axon_site")
        from trn_agent_boot.trn_boot import _ntff_profile_via_ctypes
        h = _ntff_profile_via_ctypes("/opt/axon/libaxon_pjrt.so")
        if h is not None:
            mod.set_axon_ntff_profile_hook(h)
    except Exception:
        pass


def kernel(u, A_re, A_im, B_re, B_im, C_re, C_im, D_w, output_bias):
    global LAST_RESULT, _NC_CACHE
    from concourse import bass_utils

    _ensure_axon_hooks()

    u = np.asarray(u, dtype=np.float32)
    shared16 = _host_prep(
        np.asarray(A_re), np.asarray(A_im), np.asarray(B_re), np.asarray(B_im),
        np.asarray(C_re), np.asarray(C_im), np.asarray(D_w)
    )

    if _NC_CACHE is None:
        _NC_CACHE = _build_nc()
    nc = _NC_CACHE

    in_maps = []
    for k in range(NCORES):
        u_pair = u[BLOCAL * k:BLOCAL * (k + 1)]
        ut = np.ascontiguousarray(
            u_pair.transpose(2, 0, 1).reshape(128, COLS)
        ).astype(np.float16)
        in_maps.append({"blob": np.concatenate([ut, shared16], axis=1)})

    res = bass_utils.run_bass_kernel_spmd(nc, in_maps, core_ids=list(range(NCORES)))
    LAST_RESULT = res

    y = np.empty((BATCH, T, OUT), dtype=np.float32)
    for k in range(NCORES):
        yd = res.results[k]["y"].astype(np.float32)
        y[BLOCAL * k:BLOCAL * (k + 1)] = (
            yd.reshape(OUT, BLOCAL, T).transpose(1, 2, 0)
        )
    y += np.asarray(output_bias, dtype=np.float32)
    return y


# revision 27
# speedup vs baseline: 1.0224x; 1.0224x over previous
"""Trainium2 Bass kernel for nn_BaseLinearSSM.

y[b,t] = Re(C @ x_{t+1}) + D @ u[b,t] + bias,  x_{t+1} = A x_t + B u_t  (complex A,B,C)

Strategy (v4, hybrid fp16, even/odd decimated scan):
  Host (fp64): eigendecompose A = V diag(w) V^-1, fold V into B/C:
  Bt = V^-1 B, Ct = C V.  Sort modes by |w| descending and split:

  * LONG modes (top NL=256, two 128-partition tiles, balanced ||bt_n||=1):
    decimate time by 2.  With X_k = state after consuming u_{2k+1}:
        X_k = w^2 X_{k-1} + h_k,   h_k = w f_{2k} + f_{2k+1}
    The X recurrence runs through the standard modulated-scan machinery
    (tables at 2*theta, decay rho^2) at HALF the time resolution -- this
    halves all DVE elementwise work, the only non-matmul cost.  Outputs:
        y_{2k+1} = Re(Ct X_k)          (products of X_k, lhsT Ct)
        y_{2k}   = Re(Ct w X_{k-1}) + Re(Ct Bt) u_{2k}
    i.e. odd/even y columns are STATIONARY matmuls on the demodulated
    products (lhsT Ct resp. Ct*w), written with stride-2 column APs into
    the y PSUM banks.  h is built during the PSUM->SBUF eviction: the
    scalar engine applies the per-partition scale w_re/w_im to the even
    f columns while copying (out = in*scale), so the DVE only adds.

  * SHORT modes (|w| < ~0.64): 14-lag convolution y += sum_k Re(Ct w^k
    Bt) u_{t-k} on the tensor engine (fp16 [128,128] kernels, D at lag 0).

  Numerics validated host-side: fp16 hybrid rel err ~4e-3 (gate 2e-2).
  Cores independent (weights replicated); host shards u, gathers y.
"""

import sys

import numpy as np

if "/opt/trn_rl_repo" not in sys.path:
    sys.path.insert(0, "/opt/trn_rl_repo")

BATCH, T, IN, OUT, N = 16, 2048, 128, 128, 512
NCORES = 8
BLOCAL = BATCH // NCORES  # 2
COLS = BLOCAL * T         # 4096 columns per core
BLK = 512                 # f-matmul / conv sub-block columns
DBLK = 1024               # double-block: columns per pipeline stage
NDB = COLS // DBLK        # 4 double-blocks (2 per batch element)
DPB = NDB // BLOCAL       # 2 double-blocks per batch element
HBLK = DBLK // 2          # 512 X-steps per double-block
NL = 256                  # long modes (scan path)
LT = NL // 128            # 2 long-mode partition tiles
KCONV = 12                # conv lags for short modes (incl. D at lag 0)

W_UT = COLS
W_CONV = KCONV * OUT
W_B = LT * 128
SETW = 2 * LT * HBLK      # one set's cos+sin tables at 2*theta ([128, 2048])
W_CT = LT * OUT
RHOW = LT * HBLK          # rho^2 table
BLOBW = (W_UT + W_CONV + 4 * W_B + 4 * W_CT + OUT + RHOW + DPB * SETW)

LAST_RESULT = None
_NC_CACHE = None


def _build_nc():
    from concourse import bass, mybir
    from concourse import tile

    f16 = mybir.dt.float16
    f32 = mybir.dt.float32
    op = mybir.AluOpType
    ACT = mybir.ActivationFunctionType

    nc = bass.Bass("TRN2", target_bir_lowering=False, debug=False)

    blob = nc.dram_tensor("blob", [128, BLOBW], f16, kind="ExternalInput")
    yout = nc.dram_tensor("y", [OUT, COLS], f16, kind="ExternalOutput")

    with tile.TileContext(nc) as tc:
        with (
            tc.tile_pool(name="const", bufs=1) as cpool,
            tc.tile_pool(name="f16", bufs=2) as fspool,
            tc.tile_pool(name="tmp", bufs=1) as tpool,
            tc.tile_pool(name="gp", bufs=1) as gpool,
            tc.tile_pool(name="zp", bufs=2) as zpool,
            tc.tile_pool(name="dm", bufs=2) as dpool,
            tc.tile_pool(name="ysb", bufs=2) as spool,
            tc.tile_pool(name="fps", bufs=1, space="PSUM") as fpool,
            tc.tile_pool(name="yps", bufs=2, space="PSUM") as ypool,
        ):
            blob_sb = cpool.tile([128, BLOBW], f16)
            for a, bnd in [(0, 1024), (1024, 2048), (2048, W_UT)]:
                nc.gpsimd.dma_start(blob_sb[:, a:bnd], blob[:, a:bnd])
            W0 = W_UT
            W1 = W0 + 4 * W_B                         # btr+bti+wbr+wbi
            W2 = W1 + SETW + RHOW                     # set0 tables + rho2
            W25 = W2 + W_CONV                         # convw
            W3 = W25 + 4 * W_CT + OUT                 # ctr,cti,ctwr,ctwin,k0l
            for a, bnd in [(W0, W1), (W1, W2), (W2, W25), (W25, W3),
                           (W3, W3 + SETW)]:
                nc.sync.dma_start(blob_sb[:, a:bnd], blob[:, a:bnd])

            o = [0]
            def take(w):
                s = blob_sb[:, o[0]:o[0] + w]
                o[0] += w
                return s
            ut_sb = take(W_UT)
            btr_sb = take(W_B)
            bti_sb = take(W_B)
            wbr_sb = take(W_B)
            wbi_sb = take(W_B)
            ct_set = [None] * DPB
            st_set = [None] * DPB
            ct_set[0] = take(LT * HBLK)
            st_set[0] = take(LT * HBLK)
            rho_sb = take(RHOW)
            convw = take(W_CONV)
            ctr_sb = take(W_CT)
            cti_sb = take(W_CT)       # holds -Ct.imag
            ctwr_sb = take(W_CT)
            ctwin_sb = take(W_CT)     # holds -(Ct w).imag
            k0l_sb = take(OUT)
            for k in range(1, DPB):
                ct_set[k] = take(LT * HBLK)
                st_set[k] = take(LT * HBLK)
            assert o[0] == BLOBW

            WH = LT * HBLK  # 1024: wide elementwise op width

            def emit_h(db):
                """PE: h = (w Bt) u_even + Bt u_odd directly in PSUM
                (pair-fold folded into the input matrices host-side);
                ACT: evict h to fp16 SBUF."""
                b, dbt = divmod(db, DPB)
                col0 = b * T + 2 * dbt * BLK
                ue = ut_sb[:, col0:col0 + DBLK:2]
                uo = ut_sb[:, col0 + 1:col0 + DBLK:2]
                hr16 = fspool.tile([128, WH], f16, tag="hr16")
                hi16 = fspool.tile([128, WH], f16, tag="hi16")
                for m in range(LT):
                    msl = slice(m * 128, (m + 1) * 128)
                    hpr = fpool.tile([128, HBLK], f32, tag=f"hr{m}")
                    hpi = fpool.tile([128, HBLK], f32, tag=f"hi{m}")
                    nc.tensor.matmul(hpr[:], wbr_sb[:, msl], ue, start=True, stop=False)
                    nc.tensor.matmul(hpr[:], btr_sb[:, msl], uo, start=False, stop=True)
                    nc.tensor.matmul(hpi[:], wbi_sb[:, msl], ue, start=True, stop=False)
                    nc.tensor.matmul(hpi[:], bti_sb[:, msl], uo, start=False, stop=True)
                    dst = slice(m * HBLK, (m + 1) * HBLK)
                    nc.scalar.copy(hr16[:, dst], hpr[:])
                    nc.scalar.copy(hi16[:, dst], hpi[:])
                yps = ypool.tile([128, DBLK], f32, tag="y")
                return yps, (hr16, hi16)

            def emit_conv(db, yps, sub):
                """PE: conv for one 512-col sub-block of double-block db,
                plus (on sub 1) the K0L term on even columns."""
                b, dbt = divmod(db, DPB)
                col0 = b * T + 2 * dbt * BLK
                c0 = col0 + sub * BLK
                ysl = slice(sub * BLK, (sub + 1) * BLK)
                for k in range(KCONV):
                    wk = convw[:, k * OUT:(k + 1) * OUT]
                    if dbt == 0 and sub == 0 and k > 0:
                        nc.tensor.matmul(
                            yps[:, sub * BLK + k:(sub + 1) * BLK], wk,
                            ut_sb[:, c0:c0 + BLK - k],
                            start=False, stop=False,
                        )
                    else:
                        nc.tensor.matmul(
                            yps[:, ysl], wk, ut_sb[:, c0 - k:c0 + BLK - k],
                            start=(k == 0), stop=False,
                        )
                if sub == 1:
                    # K0L on even columns -- must come after both start=True
                    # conv lag-0 matmuls have initialised the banks
                    nc.tensor.matmul(
                        yps[:, 0:DBLK:2], k0l_sb[:],
                        ut_sb[:, col0:col0 + DBLK:2],
                        start=False, stop=False,
                    )

            zprev = [None, None]

            def emit_mod(db, ev):
                """DVE: modulate h by e^{-i 2th k}."""
                hr16, hi16 = ev
                dbt = db % DPB
                ctt, stt = ct_set[dbt][:], st_set[dbt][:]
                t1 = tpool.tile([128, WH], f16, tag="t1")
                t2 = tpool.tile([128, WH], f16, tag="t2")
                t3 = tpool.tile([128, WH], f16, tag="t3")
                t4 = tpool.tile([128, WH], f16, tag="t4")
                nc.vector.tensor_tensor(t1[:], ctt, hr16[:], op=op.mult)
                nc.vector.tensor_tensor(t2[:], stt, hi16[:], op=op.mult)
                nc.vector.tensor_tensor(t3[:], ctt, hi16[:], op=op.mult)
                nc.vector.tensor_tensor(t4[:], stt, hr16[:], op=op.mult)
                return t1, t2, t3, t4

            def emit_adds_scans(db, t1, t2, t3, t4):
                dbt = db % DPB
                gr = gpool.tile([128, WH], f16, tag="gr")
                gi = gpool.tile([128, WH], f16, tag="gi")
                nc.vector.tensor_tensor(gr[:], t1[:], t2[:], op=op.add)
                nc.vector.tensor_tensor(gi[:], t3[:], t4[:], op=op.subtract)
                zr = zpool.tile([128, WH], f16, tag="zr")
                zi = zpool.tile([128, WH], f16, tag="zi")
                for m in range(LT):
                    sl = slice(m * HBLK, (m + 1) * HBLK)
                    last = m * HBLK + HBLK - 1
                    init_r = 0.0 if dbt == 0 else zprev[0][:, last:last + 1]
                    init_i = 0.0 if dbt == 0 else zprev[1][:, last:last + 1]
                    nc.vector.tensor_tensor_scan(
                        zr[:, sl], rho_sb[:, sl], gr[:, sl], init_r,
                        op0=op.mult, op1=op.add,
                    )
                    nc.vector.tensor_tensor_scan(
                        zi[:, sl], rho_sb[:, sl], gi[:, sl], init_i,
                        op0=op.mult, op1=op.add,
                    )
                zprev[0], zprev[1] = zr, zi
                return zr, zi

            def emit_demod(db, zr, zi):
                dbt = db % DPB
                ctt, stt = ct_set[dbt][:], st_set[dbt][:]
                p = dpool.tile([128, WH], f16, tag="p")
                q = dpool.tile([128, WH], f16, tag="q")
                r = dpool.tile([128, WH], f16, tag="r")
                w2 = dpool.tile([128, WH], f16, tag="w2")
                nc.vector.tensor_tensor(p[:], ctt, zr[:], op=op.mult)
                nc.vector.tensor_tensor(q[:], stt, zr[:], op=op.mult)
                nc.vector.tensor_tensor(r[:], stt, zi[:], op=op.mult)
                nc.vector.tensor_tensor(w2[:], ctt, zi[:], op=op.mult)
                pr = dpool.tile([128, WH], f16, tag="pr")
                qw = dpool.tile([128, WH], f16, tag="qw")
                nc.vector.tensor_tensor(pr[:], p[:], r[:], op=op.subtract)
                nc.vector.tensor_tensor(qw[:], q[:], w2[:], op=op.add)
                return pr, qw

            prprev = [None, None]  # (pr, qw) of previous double-block

            def emit_y(db, yps, pr, qw):
                b, dbt = divmod(db, DPB)
                col0 = b * T + 2 * dbt * BLK
                od_out = yps[:, 1:DBLK:2]
                for m in range(LT):
                    sl = slice(m * HBLK, (m + 1) * HBLK)
                    ctr = ctr_sb[:, m * OUT:(m + 1) * OUT]
                    cti = cti_sb[:, m * OUT:(m + 1) * OUT]
                    ctwr = ctwr_sb[:, m * OUT:(m + 1) * OUT]
                    ctwin = ctwin_sb[:, m * OUT:(m + 1) * OUT]
                    nc.tensor.matmul(od_out, ctr, pr[:, sl], start=False, stop=False)
                    nc.tensor.matmul(od_out, cti, qw[:, sl], start=False, stop=False)
                    sh = slice(m * HBLK, (m + 1) * HBLK - 1)
                    nc.tensor.matmul(
                        yps[:, 2:DBLK:2], ctwr, pr[:, sh], start=False, stop=False)
                    nc.tensor.matmul(
                        yps[:, 2:DBLK:2], ctwin, qw[:, sh], start=False,
                        stop=(dbt == 0 and m == LT - 1), skip_group_check=True)
                    if dbt > 0:
                        lastc = slice(m * HBLK + HBLK - 1, m * HBLK + HBLK)
                        nc.tensor.matmul(
                            yps[:, 0:1], ctwr, prprev[0][:, lastc],
                            start=False, stop=False)
                        nc.tensor.matmul(
                            yps[:, 0:1], ctwin, prprev[1][:, lastc],
                            start=False, stop=(m == LT - 1),
                            skip_group_check=True)
                prprev[0], prprev[1] = pr, qw
                ysb = spool.tile([128, DBLK], f16, tag="ysb")
                nc.scalar.copy(ysb[:], yps[:])
                nc.gpsimd.dma_start(yout[:, col0:col0 + DBLK], ysb[:])

            # software-pipelined emission over double-blocks; conv split so
            # half of it lands after y(db) on the PE queue (keeps PE warm
            # while demod products arrive).
            stage = [emit_h(0)]
            emit_conv(0, stage[0][0], 0)
            emit_conv(0, stage[0][0], 1)
            ts = emit_mod(0, stage[0][1])
            z = emit_adds_scans(0, *ts)
            for db in range(NDB):
                if db + 1 < NDB:
                    stage.append(emit_h(db + 1))
                    emit_conv(db + 1, stage[db + 1][0], 0)
                    ts_n = emit_mod(db + 1, stage[db + 1][1])
                prods = emit_demod(db, *z)
                emit_y(db, stage[db][0], *prods)
                if db + 1 < NDB:
                    emit_conv(db + 1, stage[db + 1][0], 1)
                    z = emit_adds_scans(db + 1, *ts_n)

    _legalize_multi_waits(nc)
    return nc


def _legalize_multi_waits(nc):
    """This walrus build accepts a single sync wait per instruction; split
    any multi-wait instruction into same-engine single-wait NoOps + the
    original carrying the last wait (program order chains them)."""
    import bass_rust
    from concourse import mybir

    uid = [0]
    for fn in nc.m.functions:
        for bb in fn.blocks:
            insts = bb.instructions
            new = []
            changed = False
            for inst in insts:
                si = inst.sync_info
                if si is not None and len(si.on_wait) > 1:
                    waits = list(si.on_wait)
                    for w in waits[:-1]:
                        uid[0] += 1
                        new.append(mybir.InstNoOp(
                            name=f"mwsplit-{uid[0]}",
                            engine=inst.engine,
                            ins=[], outs=[],
                            sync_info=bass_rust.SyncInfo(on_wait=[w], on_update=[]),
                        ))
                    inst.sync_info = bass_rust.SyncInfo(
                        on_wait=[waits[-1]], on_update=list(si.on_update)
                    )
                    changed = True
                new.append(inst)
            if changed:
                bb.instructions = new


def _host_prep(A_re, A_im, B_re, B_im, C_re, C_im, D_w):
    """fp64 eigendecomposition, mode sort/split, fp16 table/weight layouts."""
    A = A_re.astype(np.float64) + 1j * A_im.astype(np.float64)
    w, V = np.linalg.eig(A)
    Vinv = np.linalg.inv(V)
    Bt = Vinv @ (B_re.astype(np.float64) + 1j * B_im.astype(np.float64))
    Ct = (C_re.astype(np.float64) + 1j * C_im.astype(np.float64)) @ V
    rho_all = np.abs(w)
    order = np.argsort(-rho_all)
    li, si = order[:NL], order[NL:]

    bn = np.linalg.norm(Bt[li], axis=1)
    Btl = Bt[li] / bn[:, None]
    Ctl = Ct[:, li] * bn[None, :]
    wl = w[li]
    theta2 = 2.0 * np.angle(wl)
    rho2 = np.abs(wl) ** 2

    ws = w[si]
    convs = []
    for k in range(KCONV):
        M = ((Ct[:, si] * (ws ** k)) @ Bt[si]).real
        if k == 0:
            M = M + D_w.astype(np.float64)
        convs.append(np.ascontiguousarray(M.T, dtype=np.float16))

    # tables over X-steps k = 0..T/2-1 at angle 2*theta*(k+1)
    T2 = T // 2
    kg = np.arange(1, T2 + 1, dtype=np.float64)
    ang = np.outer(theta2, kg)  # [NL, T2]
    cost = np.cos(ang).astype(np.float16).reshape(LT, 128, T2)
    sint = np.sin(ang).astype(np.float16).reshape(LT, 128, T2)
    rho_b = np.broadcast_to(
        rho2.astype(np.float16).reshape(LT, 128, 1), (LT, 128, HBLK)
    ).transpose(1, 0, 2).reshape(128, LT * HBLK).copy()

    def setpiece(s):
        cs = cost[:, :, s * HBLK:(s + 1) * HBLK]
        ss = sint[:, :, s * HBLK:(s + 1) * HBLK]
        return [np.ascontiguousarray(cs.transpose(1, 0, 2).reshape(128, LT * HBLK)),
                np.ascontiguousarray(ss.transpose(1, 0, 2).reshape(128, LT * HBLK))]

    Ctw = Ctl * wl[None, :]
    def ct_part(M):  # [OUT, NL] -> [128, LT*OUT] lhsT layout
        MT = np.ascontiguousarray(M.T, dtype=np.float16)  # [NL, OUT]
        return np.ascontiguousarray(
            MT.reshape(LT, 128, OUT).transpose(1, 0, 2).reshape(128, LT * OUT))

    k0l = np.ascontiguousarray((Ctl @ Btl).real.T, dtype=np.float16)  # [IN, OUT]

    wBtl = wl[:, None] * Btl
    parts = [
        np.ascontiguousarray(Btl.real.T, dtype=np.float16),
        np.ascontiguousarray(Btl.imag.T, dtype=np.float16),
        np.ascontiguousarray(wBtl.real.T, dtype=np.float16),
        np.ascontiguousarray(wBtl.imag.T, dtype=np.float16),
    ]
    parts += setpiece(0)
    parts += [rho_b]
    parts += [np.concatenate(convs, axis=1)]
    parts += [ct_part(Ctl.real), ct_part(-Ctl.imag),
              ct_part(Ctw.real), ct_part(-Ctw.imag), k0l]
    for s in range(1, DPB):
        parts += setpiece(s)
    shared16 = np.concatenate(parts, axis=1)
    return shared16


def _ensure_axon_hooks():
    import types
    try:
        from antenv import axon_hooks  # noqa: F401
        return
    except ImportError:
        pass
    try:
        import antenv
        mod = types.ModuleType("antenv.axon_hooks")
        _hook = [None]
        mod.set_axon_ntff_profile_hook = lambda h: _hook.__setitem__(0, h)
        mod.get_axon_ntff_profile_hook = lambda: _hook[0]
        sys.modules["antenv.axon_hooks"] = mod
        antenv.axon_hooks = mod
        if "/root/.axon_site" not in sys.path:
            sys.path.insert(0, "/root/.axon_site")
        from trn_agent_boot.trn_boot import _ntff_profile_via_ctypes
        h = _ntff_profile_via_ctypes("/opt/axon/libaxon_pjrt.so")
        if h is not None:
            mod.set_axon_ntff_profile_hook(h)
    except Exception:
        pass


def kernel(u, A_re, A_im, B_re, B_im, C_re, C_im, D_w, output_bias):
    global LAST_RESULT, _NC_CACHE
    from concourse import bass_utils

    _ensure_axon_hooks()

    u = np.asarray(u, dtype=np.float32)
    shared16 = _host_prep(
        np.asarray(A_re), np.asarray(A_im), np.asarray(B_re), np.asarray(B_im),
        np.asarray(C_re), np.asarray(C_im), np.asarray(D_w)
    )

    if _NC_CACHE is None:
        _NC_CACHE = _build_nc()
    nc = _NC_CACHE

    in_maps = []
    for k in range(NCORES):
        u_pair = u[BLOCAL * k:BLOCAL * (k + 1)]
        ut = np.ascontiguousarray(
            u_pair.transpose(2, 0, 1).reshape(128, COLS)
        ).astype(np.float16)
        in_maps.append({"blob": np.concatenate([ut, shared16], axis=1)})

    res = bass_utils.run_bass_kernel_spmd(nc, in_maps, core_ids=list(range(NCORES)))
    LAST_RESULT = res

    y = np.empty((BATCH, T, OUT), dtype=np.float32)
    for k in range(NCORES):
        yd = res.results[k]["y"].astype(np.float32)
        y[BLOCAL * k:BLOCAL * (k + 1)] = (
            yd.reshape(OUT, BLOCAL, T).transpose(1, 2, 0)
        )
    y += np.asarray(output_bias, dtype=np.float32)
    return y
